# revision 1
# baseline (speedup 1.0000x reference)
"""Cosine-similarity multi-head attention on 8 Trainium2 NeuronCores.

Sharding: data/sequence-parallel. Core c (c = b*4 + qs) computes the full
output rows for query tokens [qs*512, (qs+1)*512) of batch b.  Each core
computes K and V for its whole batch (duplicated 4x across the cores sharing
the batch) so no collectives are needed; Q projection, attention, softmax and
the output projection are fully sharded.

Device layouts (per core):
  - x is passed transposed + chunked: xT[p, kc, n] = x[b, n, kc*128+p] (bf16)
  - Q, K are produced feature-major (qnT/knT: [head_dim on partitions, tokens])
    so the S^T = K^T_chunk.T @ Q matmuls need no transposes anywhere.
  - V is produced token-major with a ones-column appended, so one accumulating
    matmul per (head, token-chunk) yields both O^T and the softmax denominators.
  - softmax uses no max-subtraction: |logits| <= scale = 10, exp is safe in f32.
  - K normalization is folded into the exp: exp(S_raw * (1/|k_j|)) via the
    activation's per-partition scale; Q normalization (and the per-head
    temperature) is folded into qnT.
"""

import numpy as np

B, N, DIM, H, DH = 2, 2048, 1024, 16, 64
INNER = H * DH
NQ = 512            # query tokens per core
P = 128
KC = DIM // P       # 8 contraction chunks of 128
JC = N // P         # 16 key-token chunks of 128
NB = N // NQ        # 4 token blocks of 512
MAX_LOG_SCALE = float(np.log(1.0 / 0.01))

_CACHE = {}


def _build():
    if "nc" in _CACHE:
        return _CACHE["nc"]
    import concourse.bass as bass
    import concourse.bacc as bacc
    import concourse.mybir as mybir
    import concourse.tile as tile

    f32 = mybir.dt.float32
    f32r = mybir.dt.float32r
    bf16 = mybir.dt.bfloat16
    AF = mybir.ActivationFunctionType
    ALU = mybir.AluOpType

    nc = bacc.Bacc("TRN2", target_bir_lowering=False)

    xTb = nc.declare_dram_parameter("xTb", [P, KC, N], bf16, isOutput=False)
    wqb = nc.declare_dram_parameter("wqb", [P, KC, KC, P], bf16, isOutput=False)
    wkb = nc.declare_dram_parameter("wkb", [P, KC, KC, P], bf16, isOutput=False)
    wvb = nc.declare_dram_parameter("wvb", [P, 2, KC, INNER // 2], bf16, isOutput=False)
    wo2 = nc.declare_dram_parameter("wo2", [P, KC, KC, P], f32r, isOutput=False)
    bout = nc.declare_dram_parameter("bout", [P, KC], f32, isOutput=False)
    scl = nc.declare_dram_parameter("scl", [H, 1], f32, isOutput=False)
    hsmd = nc.declare_dram_parameter("hsmd", [P, KC, H], f32r, isOutput=False)
    oned = nc.declare_dram_parameter("oned", [P, JC * H], f32r, isOutput=False)
    outT = nc.declare_dram_parameter("outT", [DIM, NQ], f32, isOutput=True)

    # internal DRAM scratch
    knT_d = nc.dram_tensor("knT_d", [P, KC, N], f32r)
    fq_d = nc.dram_tensor("fq_d", [H, NQ], f32)
    fk_d = nc.dram_tensor("fk_d", [H, N], f32)

    def r(ap):
        return ap.bitcast(f32r)

    with tile.TileContext(nc) as tc:
        with (
            tc.tile_pool(name="persist", bufs=1) as pp,
        ):
            av = pp.tile([P, DH + 1, JC, H], f32r, tag="av")      # V + ones plane
            qnT = pp.tile([P, KC, NQ], f32r, tag="qnT")
            onT = pp.tile([P, KC, NQ], f32r, tag="onT")           # attn out, head pairs stacked
            hsm = pp.tile([P, KC, H], f32r, tag="hsm")
            scale_sb = pp.tile([H, 1], f32, tag="scale")
            bout_sb = pp.tile([P, KC], f32, tag="bout")
            zero_b = pp.tile([P, 1], f32, tag="zerob")

            nc.sync.dma_start(out=hsm[:], in_=hsmd[:])
            nc.sync.dma_start(out=scale_sb[:], in_=scl[:])
            nc.sync.dma_start(out=bout_sb[:], in_=bout[:])
            nc.vector.memset(zero_b[:], 0.0)
            nc.gpsimd.dma_start(out=av[:, DH].rearrange("p a b -> p (a b)"),
                                in_=oned[:])

            # ---------------- Phase A: projections ----------------
            with (
                tc.tile_pool(name="pA", bufs=2) as pa,
                tc.tile_pool(name="pAx", bufs=1) as pax,
                tc.tile_pool(name="pAs", bufs=3) as pas,
                tc.tile_pool(name="psP", bufs=3, space="PSUM") as psP,
                tc.tile_pool(name="psQ", bufs=1, space="PSUM") as psQ,
                tc.tile_pool(name="psK", bufs=1, space="PSUM") as psK,
            ):
                xq = pax.tile([P, KC, NQ], bf16, tag="xq")
                nc.sync.dma_start(out=xq[:], in_=xTb[:, :, 0:NQ])
                xt = pax.tile([P, KC, N], bf16, tag="xt")
                for _tq in range(NB):
                    nc.gpsimd.dma_start(
                        out=xt[:, :, _tq * NQ : (_tq + 1) * NQ],
                        in_=xTb[:, :, _tq * NQ : (_tq + 1) * NQ])

                # --- Q projection + norms ---
                nq = psQ.tile([H, NQ], mybir_dt_f32 := f32, tag="nq")
                for m in range(KC):
                    wt = pa.tile([P, KC, P], bf16, tag="w")
                    nc.sync.dma_start(out=wt[:], in_=wqb[:, m])
                    ps = psP.tile([P, NQ], f32, tag="pp")
                    for kc in range(KC):
                        nc.tensor.matmul(ps[:], wt[:, kc, :], xq[:, kc, :],
                                         start=(kc == 0), stop=(kc == KC - 1))
                    nc.scalar.copy(qnT[:, m, :], ps[:])
                    sq = pas.tile([P, NQ], f32r, tag="sq")
                    nc.vector.tensor_mul(sq[:], qnT[:, m, :], qnT[:, m, :])
                    nc.tensor.matmul(nq[:], hsm[:, m, :], sq[:],
                                     start=(m == 0), stop=(m == KC - 1))
                # factor_q = scale_h / sqrt(nq)
                fq = pas.tile([H, NQ], f32, tag="fq")
                nc.scalar.activation(fq[:], nq[:], AF.Sqrt, bias=zero_b[0:H, :])
                nc.vector.reciprocal(fq[:], fq[:])
                nc.vector.tensor_scalar_mul(fq[:], fq[:], scale_sb[:])
                nc.sync.dma_start(out=fq_d[:], in_=fq[:])
                for m in range(KC):
                    fqb = pas.tile([P, NQ], f32, tag="fqb")
                    nc.sync.dma_start(
                        out=fqb[0:64, :],
                        in_=fq_d[2 * m : 2 * m + 1, :].to_broadcast((64, NQ)))
                    nc.sync.dma_start(
                        out=fqb[64:P, :],
                        in_=fq_d[2 * m + 1 : 2 * m + 2, :].to_broadcast((64, NQ)))
                    nc.vector.tensor_mul(qnT[:, m, :], qnT[:, m, :], fqb[:])

                # --- K projection + norms ---
                nk = psK.tile([H, NB, NQ], f32, tag="nk")
                for m in range(KC):
                    wt = pa.tile([P, KC, P], bf16, tag="w")
                    nc.sync.dma_start(out=wt[:], in_=wkb[:, m])
                    for t in range(NB):
                        ps = psP.tile([P, NQ], f32, tag="pp")
                        for kc in range(KC):
                            nc.tensor.matmul(ps[:], wt[:, kc, :],
                                             xt[:, kc, t * NQ : (t + 1) * NQ],
                                             start=(kc == 0), stop=(kc == KC - 1))
                        kst = pas.tile([P, NQ], f32r, tag="kst")
                        nc.scalar.copy(kst[:], ps[:])
                        nc.sync.dma_start(out=knT_d[:, m, t * NQ : (t + 1) * NQ],
                                          in_=kst[:])
                        sq = pas.tile([P, NQ], f32r, tag="sq")
                        nc.vector.tensor_mul(sq[:], kst[:], kst[:])
                        nc.tensor.matmul(nk[:, t, :], hsm[:, m, :], sq[:],
                                         start=(m == 0), stop=(m == KC - 1))
                for t in range(NB):
                    fk = pas.tile([H, NQ], f32, tag="fq")
                    nc.scalar.activation(fk[:], nk[:, t, :], AF.Sqrt,
                                         bias=zero_b[0:H, :])
                    nc.vector.reciprocal(fk[:], fk[:])
                    nc.sync.dma_start(out=fk_d[:, t * NQ : (t + 1) * NQ], in_=fk[:])

                # --- V projection (token-major, into av) ---
                for fb in range(2):
                    wv = pa.tile([P, KC, INNER // 2], bf16, tag="wv")
                    nc.sync.dma_start(out=wv[:], in_=wvb[:, fb])
                    for jc in range(JC):
                        ps = psP.tile([P, NQ], f32, tag="pp")
                        for kc in range(KC):
                            nc.tensor.matmul(ps[:], xt[:, kc, jc * P : (jc + 1) * P],
                                             wv[:, kc, :],
                                             start=(kc == 0), stop=(kc == KC - 1))
                        nc.vector.tensor_copy(
                            av[:, 0:DH, jc, fb * 8 : (fb + 1) * 8]
                            .rearrange("p d h -> p h d"),
                            ps[:].rearrange("p (h d) -> p h d", d=DH))

            # ---------------- Phase B: attention ----------------
            with (
                tc.tile_pool(name="pBk", bufs=2) as pbk,
                tc.tile_pool(name="pBe", bufs=4) as pbe,
                tc.tile_pool(name="pBr", bufs=3) as pbr,
                tc.tile_pool(name="pBd", bufs=3, space="DRAM") as pbd,
                tc.tile_pool(name="psS", bufs=3, space="PSUM") as psS,
                tc.tile_pool(name="psA", bufs=2, space="PSUM") as psA,
            ):
                dnm_d = pbd.tile([H, NQ], f32, tag="dnmd", name="dnmd")
                for f in range(KC):
                    # stream + normalize K chunk: knf *= 1/|k_j| (column scale)
                    knf = pbk.tile([P, N], f32r, tag="knf")
                    nc.sync.dma_start(out=knf[:], in_=knT_d[:, f, :])
                    fkb = pbk.tile([P, N], f32, tag="fkb")
                    nc.sync.dma_start(
                        out=fkb[0:64, :],
                        in_=fk_d[2 * f : 2 * f + 1, :].to_broadcast((64, N)))
                    nc.sync.dma_start(
                        out=fkb[64:P, :],
                        in_=fk_d[2 * f + 1 : 2 * f + 2, :].to_broadcast((64, N)))
                    nc.vector.tensor_mul(knf[:], knf[:], fkb[:])
                    avps = []
                    for _half in range(2):
                        avp = psA.tile([P, NQ], f32, tag="avp", name=f"avp{_half}")
                        avps.append(avp)
                    for jc2 in range(JC // 2):
                        sp2 = []
                        for half in range(2):
                            sps = psS.tile([P, 2, NQ], f32, tag="sps",
                                           name=f"sps{half}")
                            sp2.append(sps)
                        # alternate halves so row-group-disjoint S matmuls
                        # run concurrently in the PE array
                        for q in range(2):
                            jc = 2 * jc2 + q
                            for half in range(2):
                                lo = 64 * half
                                nc.tensor.matmul(
                                    sp2[half][:, q, :],
                                    knf[lo : lo + 64, jc * P : (jc + 1) * P],
                                    qnT[lo : lo + 64, f, :],
                                    start=True, stop=True, tile_position=(lo, 0))
                        et2 = []
                        for half in range(2):
                            et = pbe.tile([P, 2, NQ], f32r, tag="et",
                                          name=f"et{half}")
                            nc.scalar.activation(et[:], sp2[half][:], AF.Exp,
                                                 bias=zero_b[:])
                            et2.append(et)
                        for q in range(2):
                            jc = 2 * jc2 + q
                            for half in range(2):
                                h = 2 * f + half
                                nc.tensor.matmul(
                                    avps[half][0:DH + 1, :],
                                    av[:, :, jc, h], et2[half][:, q, :],
                                    start=(jc == 0), stop=(jc == JC - 1))
                    for half in range(2):
                        h = 2 * f + half
                        lo = 64 * half
                        nc.vector.tensor_copy(onT[lo : lo + 64, f, :],
                                              avps[half][0:DH, :])
                        dcp = pbr.tile([1, NQ], f32, tag="dcp")
                        nc.vector.tensor_copy(dcp[:], avps[half][DH : DH + 1, :])
                        nc.sync.dma_start(out=dnm_d[h : h + 1, :], in_=dcp[:])
                    # normalize finished head-pairs in two batches so the
                    # first batch overlaps pairs 4-7 compute
                    if f in (KC // 2 - 1, KC - 1):
                        flo = 0 if f == KC // 2 - 1 else KC // 2
                        hlo = 2 * flo
                        dnm = pbr.tile([H // 2, NQ], f32, tag="dnm",
                                       name=f"dnm{flo}")
                        nc.sync.dma_start(out=dnm[:],
                                          in_=dnm_d[hlo : hlo + H // 2, :])
                        nc.vector.reciprocal(dnm[:], dnm[:])
                        nc.sync.dma_start(out=dnm_d[hlo : hlo + H // 2, :],
                                          in_=dnm[:])
                        for fb_ in range(flo, flo + KC // 2):
                            dnb = pbr.tile([P, NQ], f32, tag="dnb")
                            nc.sync.dma_start(
                                out=dnb[0:64, :],
                                in_=dnm_d[2 * fb_ : 2 * fb_ + 1, :]
                                .to_broadcast((64, NQ)))
                            nc.sync.dma_start(
                                out=dnb[64:P, :],
                                in_=dnm_d[2 * fb_ + 1 : 2 * fb_ + 2, :]
                                .to_broadcast((64, NQ)))
                            nc.vector.tensor_mul(onT[:, fb_, :],
                                                 onT[:, fb_, :], dnb[:])

            # ---------------- Phase C: output projection ----------------
            with (
                tc.tile_pool(name="pC", bufs=8) as pc,
                tc.tile_pool(name="psC", bufs=4, space="PSUM") as psC,
            ):
                wts = []
                for m in range(KC):
                    wt = pc.tile([P, KC, P], f32r, tag="wo", name=f"wo{m}")
                    nc.gpsimd.dma_start(out=wt[:], in_=wo2[:, m])
                    wts.append(wt)
                for m in range(KC):
                    wt = wts[m]
                    psa = psC.tile([P, NQ], f32, tag="po")
                    psb = psC.tile([P, NQ], f32, tag="po")
                    for g in range(KC):
                        nc.tensor.matmul(psa[:], wt[0:64, g, :], onT[0:64, g, :],
                                         start=(g == 0), stop=(g == KC - 1),
                                         tile_position=(0, 0))
                        nc.tensor.matmul(psb[:], wt[64:P, g, :], onT[64:P, g, :],
                                         start=(g == 0), stop=(g == KC - 1),
                                         tile_position=(64, 0))
                    oa = pc.tile([P, NQ], f32, tag="oa")
                    nc.vector.tensor_copy(oa[:], psa[:])
                    ot = pc.tile([P, NQ], f32, tag="ot")
                    nc.vector.scalar_tensor_tensor(
                        out=ot[:], in0=psb[:], scalar=bout_sb[:, m : m + 1],
                        in1=oa[:], op0=ALU.add, op1=ALU.add)
                    nc.sync.dma_start(out=outT[m * P : (m + 1) * P, :], in_=ot[:])

    nc.compile()
    _CACHE["nc"] = nc
    return nc


def _layout(w):
    # [DIM, C] -> [P, KC, C] with row d = kc*128 + p
    c = w.shape[1]
    return np.ascontiguousarray(w.reshape(KC, P, c).transpose(1, 0, 2))


def run(inputs, trace=False):
    import ml_dtypes
    from concourse.bass_utils import run_bass_kernel_spmd

    x = np.asarray(inputs["x"], np.float32)
    w_qkv = np.asarray(inputs["w_qkv"], np.float32)
    w_out = np.asarray(inputs["w_out"], np.float32)
    b_out = np.asarray(inputs["b_out"], np.float32)
    logit_scale = np.asarray(inputs["logit_scale"], np.float32)

    nc = _build()

    bf = ml_dtypes.bfloat16

    def _wtile(w):
        # [DIM, DIM] -> [P, KC(m), KC(kc), P]: tile (kc, m) is w[kc*128+p, m*128+q]
        return np.ascontiguousarray(
            w.reshape(KC, P, KC, P).transpose(1, 2, 0, 3))

    wqb = _wtile(w_qkv[:, 0:INNER]).astype(bf)
    wkb = _wtile(w_qkv[:, INNER : 2 * INNER]).astype(bf)
    wvb = np.ascontiguousarray(
        w_qkv[:, 2 * INNER : 3 * INNER].reshape(KC, P, 2, INNER // 2)
        .transpose(1, 2, 0, 3)).astype(bf)
    wo2 = _wtile(w_out)
    bout = np.ascontiguousarray(b_out.reshape(KC, P).T)
    scl = np.exp(np.minimum(logit_scale.reshape(H), MAX_LOG_SCALE)).astype(
        np.float32).reshape(H, 1)
    oned = np.ones((P, JC * H), np.float32)
    hsm = np.zeros((P, KC, H), np.float32)
    for f in range(KC):
        hsm[0:64, f, 2 * f] = 1.0
        hsm[64:P, f, 2 * f + 1] = 1.0

    xTb = [(_layout(np.ascontiguousarray(x[b].T)).astype(bf)) for b in range(B)]

    in_maps = []
    for c in range(8):
        b, qs = c // 4, c % 4
        xrot = np.ascontiguousarray(np.roll(xTb[b], -qs * NQ, axis=2))
        in_maps.append({
            "xTb": xrot,
            "wqb": wqb, "wkb": wkb, "wvb": wvb, "wo2": wo2,
            "bout": bout, "scl": scl, "hsmd": hsm, "oned": oned,
        })

    res = run_bass_kernel_spmd(nc, in_maps, list(range(8)), trace=trace)

    out = np.empty((B, N, DIM), np.float32)
    for c in range(8):
        b, qs = c // 4, c % 4
        out[b, qs * NQ : (qs + 1) * NQ, :] = res.results[c]["outT"].T
    return out, res


def kernel(**inputs):
    out, _ = run(inputs, trace=False)
    return out



# revision 21
# speedup vs baseline: 1.2749x; 1.2749x over previous
"""Cosine-similarity multi-head attention on 8 Trainium2 NeuronCores.

Sharding: data/sequence-parallel. Core c (c = b*4 + qs) computes the full
output rows for query tokens [qs*512, (qs+1)*512) of batch b.  Each core
computes K and V for its whole batch (duplicated 4x across the cores sharing
the batch; collectives measured too slow to be worth deduplicating).

v2 design vs the original baseline:
  - Whole attention path in bf16 (qnT/knf/et/av/onT/wo): halves SBUF+DMA
    traffic and doubles DVE throughput on the element-wise ops.
  - K projection is *lazy*: chunk f (the 128 feature rows of head pair
    2f,2f+1) is projected + normalized inside phase B's f-loop, software-
    pipelined one iteration ahead, so the PE's exp-independent work is
    interleaved with the exp-gated AV matmuls.  This keeps the tensor
    engine busy (HAM stays warm) and hides the ACT engine's softmax-exp
    stream (the true phase-B floor: 16.8M exp elements at 1 elem/cycle).
  - All norm factors (1/|q|, 1/|k|) via a DVE bit-trick rsqrt (+2 Newton
    steps), so the ACT engine never switches off the exp table set.
  - K kept normalized in SBUF ([P, KC, N] bf16) -- no DRAM K round trip.
  - w_out prefetched in bf16 at phase-B start; phase C's per-m accumulation
    runs g=0..7 in order so heads normalized in the first recip batch can
    start the out-projection while the second batch's tail normalizes.
  - softmax uses no max-subtraction: |logits| <= scale = 10, exp safe in f32.
"""

import numpy as np

B, N, DIM, H, DH = 2, 2048, 1024, 16, 64
INNER = H * DH
NQ = 512            # query tokens per core
P = 128
KC = DIM // P       # 8 feature chunks of 128
JC = N // P         # 16 key-token chunks of 128
NB = N // NQ        # 4 token blocks of 512
MAX_LOG_SCALE = float(np.log(1.0 / 0.01))
MAGIC = 0x5F3759DF

_CACHE = {}


def _build():
    if "nc" in _CACHE:
        return _CACHE["nc"]
    import concourse.bass as bass
    import concourse.bacc as bacc
    import concourse.mybir as mybir
    import concourse.tile as tile

    f32 = mybir.dt.float32
    i32 = mybir.dt.int32
    bf16 = mybir.dt.bfloat16
    AF = mybir.ActivationFunctionType
    ALU = mybir.AluOpType

    nc = bacc.Bacc("TRN2", target_bir_lowering=False)

    xTb = nc.declare_dram_parameter("xTb", [P, KC, N], bf16, isOutput=False)
    wqb = nc.declare_dram_parameter("wqb", [P, KC, KC, P], bf16, isOutput=False)
    wkb = nc.declare_dram_parameter("wkb", [P, KC, KC, P], bf16, isOutput=False)
    wvb = nc.declare_dram_parameter("wvb", [P, 2, KC, INNER // 2], bf16, isOutput=False)
    wo2 = nc.declare_dram_parameter("wo2", [P, KC, KC, P], bf16, isOutput=False)
    bout = nc.declare_dram_parameter("bout", [P, KC], f32, isOutput=False)
    # scale_h spread to rows 32*(m%4)+half, col = bank m//4 (norm-matmul layout)
    sclb = nc.declare_dram_parameter("sclb", [P, 2], f32, isOutput=False)
    outT = nc.declare_dram_parameter("outT", [DIM, NQ], f32, isOutput=True)

    # internal DRAM scratch (broadcast round trips)
    fq_d = nc.dram_tensor("fq_d", [2, P, NQ], f32)
    fk_d = nc.dram_tensor("fk_d", [KC, P, NQ], bf16)
    dnf_d = nc.dram_tensor("dnf_d", [H, NQ], f32)
    dnm_d = nc.dram_tensor("dnm_d", [H, NQ], bf16)

    def rsqrt_dve(y, x, u, out=None):
        """y/out = 1/sqrt(x) elementwise on DVE. x,y,u f32 tiles (same shape);
        if out given the final Newton step writes it (any dtype)."""
        nc.vector.tensor_scalar(
            out=y.bitcast(i32), in0=x.bitcast(i32),
            scalar1=1, scalar2=None, op0=ALU.arith_shift_right)
        nc.vector.tensor_scalar(
            out=y.bitcast(i32), in0=y.bitcast(i32),
            scalar1=-1, scalar2=MAGIC, op0=ALU.mult, op1=ALU.add)
        for it in range(2):
            nc.vector.tensor_mul(u, y, y)
            nc.vector.tensor_mul(u, u, x)
            nc.vector.tensor_scalar(
                out=u, in0=u, scalar1=-0.5, scalar2=1.5,
                op0=ALU.mult, op1=ALU.add)
            dst = y if (it == 0 or out is None) else out
            nc.vector.tensor_mul(dst, y, u)

    with tile.TileContext(nc) as tc:
        with (
            tc.tile_pool(name="persist", bufs=1) as pp,
        ):
            xt = pp.tile([P, KC, N], bf16, tag="xt")
            knf = pp.tile([P, KC, N], bf16, tag="knf")
            av = pp.tile([P, DH + 1, JC, H], bf16, tag="av")
            qnT = pp.tile([P, KC, NQ], bf16, tag="qnT")
            onT = pp.tile([P, KC, NQ], bf16, tag="onT")
            wo_sb = pp.tile([P, KC, KC, P], bf16, tag="wo")
            scale_sb = pp.tile([P, 2], f32, tag="scale")
            bout_sb = pp.tile([P, KC], f32, tag="bout")
            hsm2 = pp.tile([P, 2], bf16, tag="hsm2")

            # own token block first (Q projection input), rest via gpsimd queue
            nc.sync.dma_start(out=xt[:, :, 0:NQ], in_=xTb[:, :, 0:NQ])
            for t in range(1, NB):
                nc.gpsimd.dma_start(out=xt[:, :, t * NQ:(t + 1) * NQ],
                                    in_=xTb[:, :, t * NQ:(t + 1) * NQ])
            nc.sync.dma_start(out=scale_sb[:], in_=sclb[:])
            nc.sync.dma_start(out=bout_sb[:], in_=bout[:])
            nc.vector.memset(hsm2[:], 0.0)
            nc.vector.memset(hsm2[0:64, 0:1], 1.0)
            nc.vector.memset(hsm2[64:P, 1:2], 1.0)
            nc.vector.memset(av[:, DH, :, :], 1.0)

            # ---------------- Phase A: Q + V projections ----------------
            with (
                tc.tile_pool(name="pA", bufs=2) as pa,
                tc.tile_pool(name="pAs", bufs=3) as pas,
                tc.tile_pool(name="pAq", bufs=1) as paq,
                tc.tile_pool(name="psA", bufs=2, space="PSUM") as psA,
                tc.tile_pool(name="psN", bufs=2, space="PSUM") as psN,
            ):
                qraw = paq.tile([P, KC, NQ], f32, tag="qraw")

                nqb = [psN.tile([P, NQ], f32, tag="nqb", name=f"nqb{bk}")
                       for bk in range(2)]
                for m in range(KC):
                    wt = pa.tile([P, KC, P], bf16, tag="w")
                    nc.sync.dma_start(out=wt[:], in_=wqb[:, m])
                    ps = psA.tile([P, NQ], f32, tag="qp")
                    for kc in range(KC):
                        nc.tensor.matmul(ps[:], wt[:, kc, :], xt[:, kc, 0:NQ],
                                         start=(kc == 0), stop=(kc == KC - 1))
                    nc.scalar.copy(qraw[:, m, :], ps[:])
                    sq = pas.tile([P, NQ], bf16, tag="sq")
                    nc.vector.tensor_mul(sq[:], qraw[:, m, :], qraw[:, m, :])
                    co = 32 * (m % 4)
                    nc.tensor.matmul(nqb[m // 4][co:co + 2, :], hsm2[:], sq[:],
                                     start=True, stop=True,
                                     tile_position=(0, co))

                # fq = scale_h / |q|  (DVE rsqrt on the norm banks; rows
                # 32*(m%4)+half carry head 2m+half, other rows are junk)
                for bk in range(2):
                    fqx = pas.tile([P, NQ], f32, tag="fqx")
                    nc.vector.tensor_copy(fqx[:], nqb[bk][:])
                    fqy = pas.tile([P, NQ], f32, tag="fqy")
                    fqu = pas.tile([P, NQ], f32, tag="fqu")
                    rsqrt_dve(fqy[:], fqx[:], fqu[:])
                    nc.vector.tensor_scalar_mul(fqy[:], fqy[:],
                                                scale_sb[:, bk:bk + 1])
                    nc.sync.dma_start(out=fq_d[bk], in_=fqy[:])
                for m in range(KC):
                    ro = 32 * (m % 4)
                    fqb = pas.tile([P, NQ], f32, tag="fqb")
                    nc.sync.dma_start(
                        out=fqb[0:64, :],
                        in_=fq_d[m // 4, ro:ro + 1, :].to_broadcast((64, NQ)))
                    nc.sync.dma_start(
                        out=fqb[64:P, :],
                        in_=fq_d[m // 4, ro + 1:ro + 2, :].to_broadcast((64, NQ)))
                    nc.vector.tensor_mul(qnT[:, m, :], qraw[:, m, :], fqb[:])

                # V projection (token-major into av, bf16)
                for fb in range(2):
                    wv = pa.tile([P, KC, INNER // 2], bf16, tag="wv")
                    nc.sync.dma_start(out=wv[:], in_=wvb[:, fb])
                    for jc in range(JC):
                        ps = psA.tile([P, NQ], f32, tag="vp")
                        for kc in range(KC):
                            nc.tensor.matmul(ps[:], xt[:, kc, jc * P:(jc + 1) * P],
                                             wv[:, kc, :],
                                             start=(kc == 0), stop=(kc == KC - 1))
                        nc.vector.tensor_copy(
                            av[:, 0:DH, jc, fb * 8:(fb + 1) * 8]
                            .rearrange("p d h -> p h d"),
                            ps[:].rearrange("p (h d) -> p h d", d=DH))

            # prefetch w_out (consumed in phase C)
            nc.gpsimd.dma_start(out=wo_sb[:], in_=wo2[:])

            # ---------------- Phase B: lazy K proj + attention ----------------
            with (
                tc.tile_pool(name="pBk", bufs=2) as pbk,
                tc.tile_pool(name="pBs", bufs=3) as pbs,
                tc.tile_pool(name="pBe", bufs=4) as pbe,
                tc.tile_pool(name="pBr", bufs=2) as pbr,
                tc.tile_pool(name="psS", bufs=2, space="PSUM") as psS,
                tc.tile_pool(name="psV", bufs=1, space="PSUM") as psV,
                tc.tile_pool(name="psK", bufs=1, space="PSUM") as psK,
                tc.tile_pool(name="psNK", bufs=1, space="PSUM") as psNK,
            ):
                def kproj_gen(f):
                    """Project + normalize K chunk f into knf[:, f, :].
                    Yields 8x (2 per token block) so the caller can interleave
                    with the S/AV slots; the norm chain is emitted at drain."""
                    wt = pbk.tile([P, KC, P], bf16, tag="wk", name=f"wk{f}")
                    nc.sync.dma_start(out=wt[:], in_=wkb[:, f])
                    nkb = psNK.tile([P, NQ], f32, tag="nkb", name=f"nkb{f}")
                    for t in range(NB):
                        tsl = slice(t * NQ, (t + 1) * NQ)
                        ps = psK.tile([P, NQ], f32, tag="kp", name=f"kps{f}_{t}")
                        for kc in range(4):
                            nc.tensor.matmul(ps[:], wt[:, kc, :], xt[:, kc, tsl],
                                             start=(kc == 0), stop=False)
                        yield
                        for kc in range(4, KC):
                            nc.tensor.matmul(ps[:], wt[:, kc, :], xt[:, kc, tsl],
                                             start=False, stop=(kc == KC - 1))
                        nc.vector.tensor_copy(knf[:, f, tsl], ps[:])
                        sq = pbs.tile([P, NQ], bf16, tag="ksq")
                        nc.vector.tensor_mul(sq[:], knf[:, f, tsl], knf[:, f, tsl])
                        co = 32 * t
                        nc.tensor.matmul(nkb[co:co + 2, :], hsm2[:], sq[:],
                                         start=True, stop=True,
                                         tile_position=(0, co))
                        yield
                    # 1/|k| and fold into knf (per-column via row broadcast);
                    # rows 32t+half of the norm bank carry (block t, head
                    # 2f+half), other rows are junk
                    fkx = pbs.tile([P, NQ], f32, tag="fkx")
                    nc.vector.tensor_copy(fkx[:], nkb[:])
                    fku = pbs.tile([P, NQ], f32, tag="fku")
                    fkv = pbs.tile([P, NQ], f32, tag="fkv")
                    fk16 = pbs.tile([P, NQ], bf16, tag="fk16")
                    rsqrt_dve(fku[:], fkx[:], fkv[:], out=fk16[:])
                    nc.sync.dma_start(out=fk_d[f], in_=fk16[:])
                    for t in range(NB):
                        tsl = slice(t * NQ, (t + 1) * NQ)
                        ro = 32 * t
                        fkb = pbs.tile([P, NQ], bf16, tag="fkb")
                        nc.sync.dma_start(
                            out=fkb[0:64, :],
                            in_=fk_d[f, ro:ro + 1, :].to_broadcast((64, NQ)))
                        nc.sync.dma_start(
                            out=fkb[64:P, :],
                            in_=fk_d[f, ro + 1:ro + 2, :].to_broadcast((64, NQ)))
                        nc.vector.tensor_mul(knf[:, f, tsl], knf[:, f, tsl], fkb[:])

                def emit_av(f, jc2, ets, avps):
                    for q in range(2):
                        jc = 2 * jc2 + q
                        for half in range(2):
                            h = 2 * f + half
                            nc.tensor.matmul(
                                avps[half][0:DH + 1, :],
                                av[:, :, jc, h], ets[(jc2, half)][:, q, :],
                                start=(jc == 0), stop=(jc == JC - 1))

                def attn_f(f, ksteps):
                    avps = [psV.tile([P, NQ], f32, tag=f"avp{half}",
                                     name=f"avp{half}_{f}")
                            for half in range(2)]
                    ets = {}
                    for jc2 in range(8):
                        sp2 = []
                        for half in range(2):
                            sp2.append(psS.tile([P, 2, NQ], f32,
                                                tag="sps",
                                                name=f"sps{half}_{f}_{jc2}"))
                        for q in range(2):
                            jc = 2 * jc2 + q
                            for half in range(2):
                                lo = 64 * half
                                nc.tensor.matmul(
                                    sp2[half][:, q, :],
                                    knf[lo:lo + 64, f, jc * P:(jc + 1) * P],
                                    qnT[lo:lo + 64, f, :],
                                    start=True, stop=True, tile_position=(lo, 0))
                        for half in range(2):
                            et = pbe.tile([P, 2, NQ], bf16, tag=f"et{half}")
                            nc.scalar.activation(et[:], sp2[half][:], AF.Exp)
                            ets[(jc2, half)] = et
                        if ksteps is not None:
                            if jc2 < 4:
                                next(ksteps, None)
                                next(ksteps, None)
                            elif jc2 == 4:
                                for _ in ksteps:
                                    pass
                        if jc2 > 0:
                            emit_av(f, jc2 - 1, ets, avps)
                    emit_av(f, 7, ets, avps)
                    # attention outputs + denominators for this head pair
                    for half in range(2):
                        h = 2 * f + half
                        lo = 64 * half
                        nc.vector.tensor_copy(onT[lo:lo + 64, f, :],
                                              avps[half][0:DH, :])
                        dcp = pbr.tile([1, NQ], f32, tag="dcp")
                        nc.vector.tensor_copy(dcp[:], avps[half][DH:DH + 1, :])
                        nc.sync.dma_start(out=dnf_d[h:h + 1, :], in_=dcp[:])
                    # normalize finished head pairs in two batches
                    if f in (KC // 2 - 1, KC - 1):
                        hlo = 0 if f == KC // 2 - 1 else H // 2
                        dnm = pbr.tile([H // 2, NQ], f32, tag="dnm")
                        nc.sync.dma_start(out=dnm[:], in_=dnf_d[hlo:hlo + 8, :])
                        nc.vector.reciprocal(dnm[:], dnm[:])
                        dnr = pbr.tile([H // 2, NQ], bf16, tag="dnr")
                        nc.vector.tensor_copy(dnr[:], dnm[:])
                        nc.sync.dma_start(out=dnm_d[hlo:hlo + 8, :], in_=dnr[:])
                        for ff in range(hlo // 2, hlo // 2 + 4):
                            dnb = pbr.tile([P, NQ], bf16, tag="dnb")
                            nc.sync.dma_start(
                                out=dnb[0:64, :],
                                in_=dnm_d[2 * ff:2 * ff + 1, :]
                                .to_broadcast((64, NQ)))
                            nc.sync.dma_start(
                                out=dnb[64:P, :],
                                in_=dnm_d[2 * ff + 1:2 * ff + 2, :]
                                .to_broadcast((64, NQ)))
                            nc.vector.tensor_mul(onT[:, ff, :],
                                                 onT[:, ff, :], dnb[:])

                # prologue: K chunk 0 eagerly
                for _ in kproj_gen(0):
                    pass
                for f in range(KC):
                    g = kproj_gen(f + 1) if f < KC - 1 else None
                    attn_f(f, g)
                    if g is not None:
                        for _ in g:
                            pass

            # ---------------- Phase C: output projection ----------------
            with (
                tc.tile_pool(name="pC", bufs=3) as pc,
                tc.tile_pool(name="psC", bufs=2, space="PSUM") as psC,
            ):
                for m in range(KC):
                    psa = psC.tile([P, NQ], f32, tag="poa")
                    psb = psC.tile([P, NQ], f32, tag="pob")
                    for g in range(KC):
                        nc.tensor.matmul(psa[:], wo_sb[0:64, m, g, :],
                                         onT[0:64, g, :],
                                         start=(g == 0), stop=(g == KC - 1),
                                         tile_position=(0, 0))
                        nc.tensor.matmul(psb[:], wo_sb[64:P, m, g, :],
                                         onT[64:P, g, :],
                                         start=(g == 0), stop=(g == KC - 1),
                                         tile_position=(64, 0))
                    oa = pc.tile([P, NQ], f32, tag="oa")
                    nc.vector.tensor_copy(oa[:], psa[:])
                    ot = pc.tile([P, NQ], f32, tag="ot")
                    nc.vector.scalar_tensor_tensor(
                        out=ot[:], in0=psb[:], scalar=bout_sb[:, m:m + 1],
                        in1=oa[:], op0=ALU.add, op1=ALU.add)
                    nc.sync.dma_start(out=outT[m * P:(m + 1) * P, :], in_=ot[:])

    nc.compile()
    _CACHE["nc"] = nc
    return nc


def _layout(w):
    # [DIM, C] -> [P, KC, C] with row d = kc*128 + p
    c = w.shape[1]
    return np.ascontiguousarray(w.reshape(KC, P, c).transpose(1, 0, 2))


def run(inputs, trace=False):
    import ml_dtypes
    from concourse.bass_utils import run_bass_kernel_spmd

    x = np.asarray(inputs["x"], np.float32)
    w_qkv = np.asarray(inputs["w_qkv"], np.float32)
    w_out = np.asarray(inputs["w_out"], np.float32)
    b_out = np.asarray(inputs["b_out"], np.float32)
    logit_scale = np.asarray(inputs["logit_scale"], np.float32)

    nc = _build()

    bf = ml_dtypes.bfloat16

    def _wtile(w):
        # [DIM, DIM] -> [P, KC(m), KC(kc), P]: tile (kc, m) is w[kc*128+p, m*128+q]
        return np.ascontiguousarray(
            w.reshape(KC, P, KC, P).transpose(1, 2, 0, 3))

    wqb = _wtile(w_qkv[:, 0:INNER]).astype(bf)
    wkb = _wtile(w_qkv[:, INNER:2 * INNER]).astype(bf)
    wvb = np.ascontiguousarray(
        w_qkv[:, 2 * INNER:3 * INNER].reshape(KC, P, 2, INNER // 2)
        .transpose(1, 2, 0, 3)).astype(bf)
    wo2 = _wtile(w_out).astype(bf)
    bout = np.ascontiguousarray(b_out.reshape(KC, P).T)
    scale = np.exp(np.minimum(logit_scale.reshape(H), MAX_LOG_SCALE)).astype(
        np.float32)
    sclb = np.zeros((P, 2), np.float32)
    for h in range(H):
        m, half = h // 2, h % 2
        sclb[32 * (m % 4) + half, m // 4] = scale[h]

    xTb = [(_layout(np.ascontiguousarray(x[b].T)).astype(bf)) for b in range(B)]

    in_maps = []
    for c in range(8):
        b, qs = c // 4, c % 4
        xrot = np.ascontiguousarray(np.roll(xTb[b], -qs * NQ, axis=2))
        in_maps.append({
            "xTb": xrot,
            "wqb": wqb, "wkb": wkb, "wvb": wvb, "wo2": wo2,
            "bout": bout, "sclb": sclb,
        })

    res = run_bass_kernel_spmd(nc, in_maps, list(range(8)), trace=trace)

    out = np.empty((B, N, DIM), np.float32)
    for c in range(8):
        b, qs = c // 4, c % 4
        out[b, qs * NQ:(qs + 1) * NQ, :] = res.results[c]["outT"].T
    return out, res


def kernel(**inputs):
    out, _ = run(inputs, trace=False)
    return out
